# revision 1
# baseline (speedup 1.0000x reference)
"""Banded (sliding-window) GQA attention block on 8 trn2 cores.

Sharding: 8 cores = batch(4) x seq-halves(2). Each core computes 1024
queries for one batch element with a 127-position K/V halo on each side.
All layouts are transposed ([feature, seq]) so the tensor engine contracts
naturally; RoPE even/odd lanes are split into separate tensors (same
partitions) so the rotation is full-width DVE work.

Attention is computed in S.T layout via diagonal key-chunk blocks
[128 keys x 384 queries]; band masking is accumulated into PSUM with an
identity matmul; softmax denominators come from an appended ones-column
in V (so no max-subtraction: scores are small enough that raw exp fits
comfortably in f32).
"""

import sys

sys.path.insert(0, "/opt/trn_rl_repo")

import numpy as np

import concourse.bass as bass
from concourse import bacc
import concourse.mybir as mybir
import concourse.tile as tile
from concourse.bass_utils import run_bass_kernel_spmd
from concourse.masks import make_identity

B, S, D = 4, 2048, 1024
H, KVH, HD = 16, 2, 64
W, HWD = 255, 127
SL = S // 2              # local queries per core
U = SL + 2 * HWD + 2     # 1280 padded key columns (1278 + 2 round-up)
UQ = U + 256             # 1536: query tensors padded 128 each side
NCH = U // 128           # 10 key chunks
NEG = -1.0e30

f32 = mybir.dt.float32
f32r = mybir.dt.float32r
bf16 = mybir.dt.bfloat16


def build_nc():
    nc = bacc.Bacc("TRN2")
    dp = nc.declare_dram_parameter
    xT = dp("xT", [D, U], f32r, isOutput=False)
    wqe = dp("wqe", [D, 512], f32r, isOutput=False)
    wqo = dp("wqo", [D, 512], f32r, isOutput=False)
    wke = dp("wke", [D, 256], f32r, isOutput=False)
    wko = dp("wko", [D, 256], f32r, isOutput=False)
    wv = dp("wv", [D, 128], f32r, isOutput=False)
    wo = dp("wo", [D, D], f32r, isOutput=False)
    bqe = dp("bqe", [1, 512], f32r, isOutput=False)
    bqo = dp("bqo", [1, 512], f32r, isOutput=False)
    bke = dp("bke", [1, 256], f32r, isOutput=False)
    bko = dp("bko", [1, 256], f32r, isOutput=False)
    bvb = dp("bvb", [1, 128], f32r, isOutput=False)
    bob = dp("bob", [1, D], f32r, isOutput=False)
    cosq = dp("cosq", [128, U], f32, isOutput=False)
    sinq = dp("sinq", [128, U], f32, isOutput=False)
    cosk = dp("cosk", [128, U], f32, isOutput=False)
    sink = dp("sink", [128, U], f32, isOutput=False)
    maskT = dp("maskT", [128, 384], f32r, isOutput=False)
    out = dp("out", [SL, D], f32, isOutput=True)

    NB = [(0, 512), (512, 512), (1024, 256)]  # N-blocks over U

    with tile.TileContext(nc) as tc:
        with (
            nc.allow_low_precision(reason="f32r tiles are 4-byte; elementwise ops only"),
            tc.tile_pool(name="persist", bufs=1) as pe,
        ):
            # ---- persistent SBUF ----
            ident_f = pe.tile([128, 128], f32, tag="identf")
            make_identity(nc, ident_f)
            ident = pe.tile([128, 128], f32r, tag="ident")
            nc.vector.tensor_copy(ident[:], ident_f[:])
            ones_f = pe.tile([1, 512], f32, tag="onesf")
            nc.vector.memset(ones_f[:], 1.0)
            ones = pe.tile([1, 512], f32r, tag="ones")
            nc.vector.tensor_copy(ones[:], ones_f[:])
            mask_t = pe.tile([128, 384], f32r, tag="mask")
            nc.gpsimd.dma_start(mask_t[:], maskT[:])
            qte = [pe.tile([128, UQ], f32r, tag=f"qte{g}", name=f"qte{g}") for g in range(4)]
            qto = [pe.tile([128, UQ], f32r, tag=f"qto{g}", name=f"qto{g}") for g in range(4)]
            ktr_e = [pe.tile([128, U], f32r, tag=f"kte{b2}", name=f"kte{b2}") for b2 in range(2)]
            ktr_o = [pe.tile([128, U], f32r, tag=f"kto{b2}", name=f"kto{b2}") for b2 in range(2)]
            vaug = [pe.tile([128, 65 * NCH], bf16, tag=f"vaug{k}", name=f"vaug{k}") for k in range(2)]
            bo_sb = pe.tile([1, D], f32r, tag="bo")
            nc.gpsimd.dma_start(bo_sb[:], bob[:])

            for k in range(2):
                nc.vector.memset(vaug[k][:], 1.0)
            zf = pe.tile([128, 128], f32, tag="zf")
            nc.vector.memset(zf[:], 0.0)
            for g in range(4):
                nc.vector.tensor_copy(qte[g][:, 0:128], zf[:])
                nc.vector.tensor_copy(qte[g][:, UQ - 128 : UQ], zf[:])
                nc.vector.tensor_copy(qto[g][:, 0:128], zf[:])
                nc.vector.tensor_copy(qto[g][:, UQ - 128 : UQ], zf[:])

            # ================= phase A: projections + rope =================
            with (
                tc.tile_pool(name="proj_in", bufs=1) as pin,
                tc.tile_pool(name="tmp", bufs=1) as ptmp,
                tc.tile_pool(name="qps", bufs=2, space="PSUM") as qps,
                tc.tile_pool(name="vps", bufs=2, space="PSUM") as vps,
            ):
                xts = [pin.tile([128, U], f32r, tag=f"x{i}", name=f"x{i}") for i in range(8)]
                for i in range(8):
                    nc.gpsimd.dma_start(xts[i][:], xT[128 * i : 128 * i + 128, :])
                wke_s = [pin.tile([128, 256], f32r, tag=f"wke{i}", name=f"wke{i}") for i in range(8)]
                wko_s = [pin.tile([128, 256], f32r, tag=f"wko{i}", name=f"wko{i}") for i in range(8)]
                wv_s = [pin.tile([128, 128], f32r, tag=f"wv{i}", name=f"wv{i}") for i in range(8)]
                for i in range(8):
                    sl = slice(128 * i, 128 * i + 128)
                    nc.gpsimd.dma_start(wke_s[i][:], wke[sl, :])
                    nc.gpsimd.dma_start(wko_s[i][:], wko[sl, :])
                    nc.gpsimd.dma_start(wv_s[i][:], wv[sl, :])
                cq = pin.tile([128, U], f32, tag="cq")
                sq = pin.tile([128, U], f32, tag="sq")
                ck = pin.tile([128, U], f32, tag="ck")
                sk = pin.tile([128, U], f32, tag="sk")
                nc.gpsimd.dma_start(cq[:], cosq[:])
                nc.gpsimd.dma_start(sq[:], sinq[:])
                nc.gpsimd.dma_start(ck[:], cosk[:])
                nc.gpsimd.dma_start(sk[:], sink[:])
                be_s = pin.tile([1, 512], f32r, tag="bqe")
                bo_s2 = pin.tile([1, 512], f32r, tag="bqo")
                bke_s = pin.tile([1, 256], f32r, tag="bke")
                bko_s = pin.tile([1, 256], f32r, tag="bko")
                bv_s = pin.tile([1, 128], f32r, tag="bv")
                nc.gpsimd.dma_start(be_s[:], bqe[:])
                nc.gpsimd.dma_start(bo_s2[:], bqo[:])
                nc.gpsimd.dma_start(bke_s[:], bke[:])
                nc.gpsimd.dma_start(bko_s[:], bko[:])
                nc.gpsimd.dma_start(bv_s[:], bvb[:])

                def proj(ps, wtiles, wsl, btile, bsl, nrows):
                    # ps[(0:nrows), 0:U] = (w slice).T @ xT + bias
                    for n0, nw in NB:
                        for kc in range(8):
                            nc.tensor.matmul(
                                ps[0:nrows, n0 : n0 + nw],
                                wtiles[kc][:, wsl],
                                xts[kc][:, n0 : n0 + nw],
                                start=(kc == 0),
                                stop=False,
                            )
                        nc.tensor.matmul(
                            ps[0:nrows, n0 : n0 + nw],
                            btile[0:1, bsl],
                            ones[0:1, 0:nw],
                            start=False,
                            stop=True,
                        )

                def rope(ps_e, ps_o, dst_e, dst_o, c, s, nrows, width, dcol):
                    t1 = ptmp.tile([128, U], f32, tag="t1")
                    t2 = ptmp.tile([128, U], f32, tag="t2")
                    r = slice(0, nrows)
                    w = slice(0, width)
                    cc = c[r, 0:width]
                    ss = s[r, 0:width]
                    mult, add, sub = (
                        mybir.AluOpType.mult,
                        mybir.AluOpType.add,
                        mybir.AluOpType.subtract,
                    )
                    nc.vector.tensor_tensor(t1[r, w], ps_e[r, w], cc, mult)
                    nc.vector.tensor_tensor(t2[r, w], ps_o[r, w], ss, mult)
                    nc.vector.tensor_tensor(
                        dst_e[r, dcol : dcol + width], t1[r, w], t2[r, w], sub
                    )
                    t3 = ptmp.tile([128, U], f32, tag="t1")
                    t4 = ptmp.tile([128, U], f32, tag="t2")
                    nc.vector.tensor_tensor(t3[r, w], ps_e[r, w], ss, mult)
                    nc.vector.tensor_tensor(t4[r, w], ps_o[r, w], cc, mult)
                    nc.vector.tensor_tensor(
                        dst_o[r, dcol : dcol + width], t3[r, w], t4[r, w], add
                    )

                for b2 in range(2):
                    ps_e = qps.tile([128, U], f32, tag="qp")
                    ps_o = qps.tile([128, U], f32, tag="qp")
                    proj(ps_e, wke_s, slice(128 * b2, 128 * b2 + 128), bke_s,
                         slice(128 * b2, 128 * b2 + 128), 128)
                    proj(ps_o, wko_s, slice(128 * b2, 128 * b2 + 128), bko_s,
                         slice(128 * b2, 128 * b2 + 128), 128)
                    rope(ps_e, ps_o, ktr_e[b2], ktr_o[b2], ck, sk, 128, U, 0)

                # V projection (f32 for accuracy), ones column kept at 1.0
                for sti in range(NCH):
                    vp = vps.tile([128, 128], f32, tag="vp")
                    ssl = slice(128 * sti, 128 * sti + 128)
                    for kc in range(8):
                        nc.tensor.matmul(
                            vp[:], xts[kc][:, ssl], wv_s[kc][:], start=(kc == 0),
                            stop=False,
                        )
                    nc.tensor.matmul(
                        vp[:], ones[0:1, 0:128], bv_s[:], start=False, stop=True
                    )
                    for k in range(2):
                        nc.scalar.copy(
                            vaug[k][:, 65 * sti : 65 * sti + 64],
                            vp[:, 64 * k : 64 * k + 64],
                        )

                for gh in range(2):
                    wqe_s = [pin.tile([128, 256], f32r, tag=f"wqe{i}", name=f"wqeh{i}") for i in range(8)]
                    wqo_s = [pin.tile([128, 256], f32r, tag=f"wqo{i}", name=f"wqoh{i}") for i in range(8)]
                    for i in range(8):
                        sl = slice(128 * i, 128 * i + 128)
                        nc.gpsimd.dma_start(wqe_s[i][:], wqe[sl, 256 * gh : 256 * gh + 256])
                        nc.gpsimd.dma_start(wqo_s[i][:], wqo[sl, 256 * gh : 256 * gh + 256])
                    for g in (2 * gh, 2 * gh + 1):
                        ps_e = qps.tile([128, U], f32, tag="qp")
                        ps_o = qps.tile([128, U], f32, tag="qp")
                        proj(ps_e, wqe_s, slice(128 * (g % 2), 128 * (g % 2) + 128), be_s,
                             slice(128 * g, 128 * g + 128), 128)
                        proj(ps_o, wqo_s, slice(128 * (g % 2), 128 * (g % 2) + 128), bo_s2,
                             slice(128 * g, 128 * g + 128), 128)
                        rope(ps_e, ps_o, qte[g], qto[g], cq, sq, 128, U, 128)

            # ============ phase B: scores -> exp -> PV -> normalize ============
            with tc.tile_pool(name="pattn", bufs=1) as pattn:
              attn = [pattn.tile([128, U], f32r, tag=f"attn{t}", name=f"attn{t}") for t in range(8)]
              with (
                tc.tile_pool(name="spool", bufs=2, space="PSUM") as spool,
                tc.tile_pool(name="ppool", bufs=3) as ppool,
                tc.tile_pool(name="npool", bufs=3) as npool,
                tc.tile_pool(name="ppv", bufs=3, space="PSUM") as ppv,
                tc.tile_pool(name="prb", bufs=1, space="PSUM") as prb,
                tc.tile_pool(name="pqx", bufs=1) as pqx,
              ):
                for h in range(H):
                    kv = h // 8
                    gq = h // 4
                    if h % 4 == 3:
                        qxe = pqx.tile([32, UQ], f32r, tag="qxe", name="qxe")
                        qxo = pqx.tile([32, UQ], f32r, tag="qxo", name="qxo")
                        nc.vector.tensor_copy(qxe[:], qte[gq][96:128, :])
                        nc.vector.tensor_copy(qxo[:], qto[gq][96:128, :])
                        qe_t, qo_t, rq = qxe, qxo, slice(0, 32)
                    else:
                        qe_t, qo_t, rq = qte[gq], qto[gq], slice(32 * (h % 4), 32 * (h % 4) + 32)
                    pv_ps = {}
                    for m in range(3):
                        pv_ps[m] = ppv.tile([128, 512], f32, tag="pv", name=f"pv{m}")

                    pts = {}
                    for p in range(NCH // 2):
                        sp = spool.tile([128, 1024], f32, tag="sc")
                        pt = ppool.tile([128, 768], bf16, tag="pt")
                        pts[p] = pt
                        for half in range(2):
                            c = 2 * p + half
                            c0 = 128 * c
                            col = 512 * half
                            nc.tensor.matmul(
                                sp[:, col : col + 384],
                                ktr_e[kv][rq, c0 : c0 + 128],
                                qe_t[rq, c0 : c0 + 384],
                                start=True, stop=False,
                            )
                            nc.tensor.matmul(
                                sp[:, col : col + 384],
                                ktr_o[kv][rq, c0 : c0 + 128],
                                qo_t[rq, c0 : c0 + 384],
                                start=False, stop=False,
                            )
                            nc.tensor.matmul(
                                sp[:, col : col + 384],
                                ident[:],
                                mask_t[:],
                                start=False, stop=True,
                            )
                        sview = sp[:].rearrange("p (b x) -> p b x", b=2)[:, :, 0:384]
                        pview = pt[:].rearrange("p (b x) -> p b x", b=2)
                        nc.scalar.activation(
                            pview, sview, mybir.ActivationFunctionType.Exp
                        )
                        lo = max(0, 2 * p - 1)
                        hi = 2 * p if p < NCH // 2 - 1 else NCH - 1
                        for j in range(lo, hi + 1):
                            for c in (j - 1, j, j + 1):
                                if c < 0 or c >= NCH:
                                    continue
                                pt_c = pts[c // 2]
                                base = 384 * (c % 2) + 128 * (j - c + 1)
                                nc.tensor.matmul(
                                    pv_ps[j // 4][0:65, 128 * (j % 4) : 128 * (j % 4) + 128],
                                    vaug[kv][:, 65 * c : 65 * c + 65],
                                    pt_c[:, base : base + 128],
                                    start=(c == max(0, j - 1)),
                                    stop=(c == min(NCH - 1, j + 1)),
                                )

                    # normalize: attn[t] rows = out.T rows for this head
                    t = h // 2
                    r0 = 64 * (h % 2)
                    for m in range(3):
                        wdt = 512 if m < 2 else 256
                        rd = npool.tile([1, 512], f32r, tag="rd")
                        nc.vector.reciprocal(rd[0:1, 0:wdt], pv_ps[m][64:65, 0:wdt])
                        rb_ps = prb.tile([128, 512], f32, tag="rb")
                        nc.tensor.matmul(
                            rb_ps[0:64, 0:wdt],
                            ones[0:1, 0:64],
                            rd[0:1, 0:wdt],
                            start=True, stop=True,
                        )
                        rb_sb = npool.tile([64, 512], f32, tag="rbs")
                        nc.scalar.copy(rb_sb[0:64, 0:wdt], rb_ps[0:64, 0:wdt])
                        nc.vector.tensor_tensor(
                            attn[t][r0 : r0 + 64, 512 * m : 512 * m + wdt],
                            pv_ps[m][0:64, 0:wdt],
                            rb_sb[0:64, 0:wdt],
                            mybir.AluOpType.mult,
                        )

              # ================= phase C: output projection =================
              with (
                  tc.tile_pool(name="wop", bufs=1) as pwo,
                  tc.tile_pool(name="oout", bufs=3) as pou,
                  tc.tile_pool(name="ops", bufs=2, space="PSUM") as ops,
              ):
                  wo_s = [pwo.tile([128, D], f32r, tag=f"wo{i}", name=f"wo{i}") for i in range(8)]
                  for i in range(8):
                      nc.gpsimd.dma_start(wo_s[i][:], wo[128 * i : 128 * i + 128, :])
                  for tq in range(8):
                      q0 = 127 + 128 * tq
                      for nb in range(2):
                          op = ops.tile([128, 512], f32, tag="op")
                          for kc in range(8):
                              nc.tensor.matmul(
                                  op[:],
                                  attn[kc][:, q0 : q0 + 128],
                                  wo_s[kc][:, 512 * nb : 512 * nb + 512],
                                  start=(kc == 0), stop=False,
                              )
                          nc.tensor.matmul(
                              op[:],
                              ones[0:1, 0:128],
                              bo_sb[0:1, 512 * nb : 512 * nb + 512],
                              start=False, stop=True,
                          )
                          ot = pou.tile([128, 512], f32, tag="ot")
                          nc.scalar.copy(ot[:], op[:])
                          nc.sync.dma_start(
                              out[128 * tq : 128 * tq + 128, 512 * nb : 512 * nb + 512],
                              ot[:],
                          )
    nc.finalize()
    return nc


_PERM_QE = np.array(
    [(4 * g + a) * 64 + 2 * i for g in range(4) for a in range(4) for i in range(32)]
)
_PK = [np.array([kv * 64 + 2 * i for i in range(32)]) for kv in range(2)]


def make_inputs(x, freqs_cis, w_q, b_q, w_k, b_k, w_v, b_v, w_o, b_o):
    cos = np.asarray(freqs_cis[..., 0], dtype=np.float32)  # (S, 32)
    sin = np.asarray(freqs_cis[..., 1], dtype=np.float32)
    x = np.asarray(x, dtype=np.float32)
    maskT = np.full((128, 384), NEG, dtype=np.float32)
    for k in range(128):
        maskT[k, k + 1 : k + 256] = 0.0
    common = dict(
        wqe=np.ascontiguousarray(w_q[:, _PERM_QE]),
        wqo=np.ascontiguousarray(w_q[:, _PERM_QE + 1]),
        wke=np.concatenate([np.tile(w_k[:, _PK[kv]], (1, 4)) for kv in range(2)], 1),
        wko=np.concatenate([np.tile(w_k[:, _PK[kv] + 1], (1, 4)) for kv in range(2)], 1),
        wv=np.ascontiguousarray(w_v),
        wo=np.ascontiguousarray(w_o),
        bqe=b_q[_PERM_QE][None, :].astype(np.float32),
        bqo=b_q[_PERM_QE + 1][None, :].astype(np.float32),
        bke=np.concatenate([np.tile(b_k[_PK[kv]], 4) for kv in range(2)])[None, :].astype(np.float32),
        bko=np.concatenate([np.tile(b_k[_PK[kv] + 1], 4) for kv in range(2)])[None, :].astype(np.float32),
        bvb=np.asarray(b_v, dtype=np.float32)[None, :],
        bob=np.asarray(b_o, dtype=np.float32)[None, :],
        maskT=maskT,
    )
    maps = []
    for c in range(8):
        b, hf = c // 2, c % 2
        s0 = SL * hf
        pos = s0 - HWD + np.arange(U)
        valid = (pos >= 0) & (pos < S)
        pc = np.clip(pos, 0, S - 1)
        xTc = np.where(valid[None, :], x[b][pc].T, 0.0).astype(np.float32)
        ckc = np.tile(cos[pc].T, (4, 1)).astype(np.float32)
        skc = np.tile(sin[pc].T, (4, 1)).astype(np.float32)
        cq = np.tile(cos[pc].T, (4, 1)).astype(np.float32)
        sq = np.tile(sin[pc].T, (4, 1)).astype(np.float32)
        m = dict(common)
        m.update(xT=xTc, cosq=cq, sinq=sq, cosk=ckc, sink=skc)
        maps.append(m)
    return maps


_NC_CACHE = {}


def kernel(x, freqs_cis, w_q, b_q, w_k, b_k, w_v, b_v, w_o, b_o):
    if "nc" not in _NC_CACHE:
        _NC_CACHE["nc"] = build_nc()
    nc = _NC_CACHE["nc"]
    maps = make_inputs(
        np.asarray(x), np.asarray(freqs_cis), np.asarray(w_q), np.asarray(b_q),
        np.asarray(w_k), np.asarray(b_k), np.asarray(w_v), np.asarray(b_v),
        np.asarray(w_o), np.asarray(b_o),
    )
    res = run_bass_kernel_spmd(nc, maps, list(range(8))).results
    full = np.empty((B, S, D), np.float32)
    for c in range(8):
        b, hf = c // 2, c % 2
        full[b, SL * hf : SL * (hf + 1), :] = res[c]["out"]
    return full

